# revision 1
# baseline (speedup 1.0000x reference)
"""AMRPA attention wrapper kernel for 8 TRN2 NeuronCores.

Sharding: data-parallel over (batch, seq-half). Core c handles batch b=c//2,
query rows [h*1024, (h+1)*1024) with h=c%2. k/v projections are split across
the core pair by KEY half: each core projects k/v only for its own sequence
rows (which equal its query rows), so only hsq ([H, SQ]) is ever loaded --
the full hsT is not needed. The halves are exchanged with pair AllGathers
(key-major concat in rank order = global key order).

Math (per core, Sq=1024 query rows, S=2048 keys, H=1024):
  qT = Wq^T hsq, kT_own = (Wk/sqrt(H))^T hsq, v_own = hsq^T Wv
  AllGather(v_own) -> v [S, H];  AllGather(kT_own) -> kT [H, S]
  g = sigmoid(q . w_gate)                                   (per row)
  mvT = v^T paT   (fp8 DoubleRow; paT pre-scaled by PA_SCALE on host)
  tfT = wm8^T mvT (fp8 DoubleRow; wm8 = Wm e^-0.5 * WM_SCALE)
  qhatT = qT + (g / (PA_SCALE*WM_SCALE)) * tfT   (memory bias folded into q)
  logits = qhat kT; probs = exp(logits); context = (probs v)/rowsum(probs)

The memory path (mvT/tfT) runs in fp8e4 with DoubleRow perf mode (2 k-tiles
per matmul): the memory bias contributes <1% of the output magnitude, so fp8
error there is negligible. Main path stays bf16 (fp32 PSUM accumulation).
"""

import math
import sys

import numpy as np
import ml_dtypes

import concourse.bass as bass
import concourse.mybir as mybir
import concourse.tile as tile
from concourse.bass_utils import run_bass_kernel_spmd
from concourse.masks import make_identity
from concourse.vector_clock import ScopedClock

BF16 = mybir.dt.bfloat16
F8 = mybir.dt.float8e4
F32 = mybir.dt.float32

B, S, H = 4, 2048, 1024
SQ = S // 2  # query rows per core
N_CORES = 8
NT_H = H // 128   # 8 partition tiles over hidden dim
NT_S = S // 128   # 16 partition tiles over sequence
NT_SP = NT_S // 2  # 8 k-tile PAIRS (fp8 DoubleRow)
NT_DP = NT_H // 2  # 4 d-tile PAIRS (fp8 DoubleRow)
NT_Q = SQ // 128  # 8 query row tiles per core
NC_S = S // 512   # 4 free-dim chunks over sequence
NC_Q = SQ // 512  # 2 free-dim chunks over query rows
NC_H = H // 512   # 2 free-dim chunks over hidden

PA_SCALE = 1024.0  # paT pre-scale so fp8e4 sees O(1) values
WM_SCALE = 32.0    # Wm pre-scale for fp8e4 range
G_SCALE = 1.0 / (PA_SCALE * WM_SCALE)  # folded into the gate broadcast

# ---------------------------------------------------------------------------
# Workaround: this walrus build allows only one sync-wait on a Drain
# instruction; Tile's kernel-tail drain carries one wait per DMA-HW
# semaphore. Split the tail drain into a chain of single-wait drains.
# ---------------------------------------------------------------------------


def _patched_drain_and_barrier(self, tick_clock, wait_clock):
    nc = self.nc
    drain_inst = nc.sync.drain()
    wait_clock.add_sem_waits(
        drain_inst.ins, ScopedClock({None: tick_clock.global_clock})
    )
    si = drain_inst.ins.sync_info
    if si is not None and si.on_wait and len(si.on_wait) > 1:
        waits = list(si.on_wait)
        si.on_wait = waits[:1]
        for w in waits[1:]:
            d = nc.sync.drain()
            dsi = d.ins.sync_info
            if dsi is None:
                d.ins.sync_info = mybir.SyncInfo(on_wait=[w], on_update=[])
            else:
                dsi.on_wait = [w]

    nc.all_engine_barrier()
    assert self.sems is not None
    popped = nc._tile_sem_poison_stack.pop()
    assert popped is self._sem_poison
    nc.clear_and_free_semaphores(list(self.sems.allocated().values()))
    nc.all_engine_barrier()


tile.TileContext._drain_and_barrier = _patched_drain_and_barrier


def _split_multi_wait_instructions(nc: bass.Bass):
    """Walrus here allows only one sync-wait per instruction. Move extra
    waits onto injected same-engine NoOps placed just before the owner."""
    bbs = [(bb, list(bb.instructions)) for f in nc.m.functions for bb in f.blocks]
    new_lists = []
    for bb, insts in bbs:
        new_list = []
        for inst in insts:
            si = inst.sync_info
            if si is not None and si.on_wait and len(si.on_wait) > 1:
                waits = list(si.on_wait)
                for w in waits[:-1]:
                    bi = nc.engines[inst.engine].nop(nofuse=True)
                    ni = bi.ins
                    ni.sync_info = mybir.SyncInfo(on_wait=[w], on_update=[])
                    new_list.append(ni)
                si.on_wait = [waits[-1]]
            new_list.append(inst)
        new_lists.append((bb, new_list))
    for bb, nl in new_lists:
        bb.instructions = nl


def build_nc() -> bass.Bass:
    nc = bass.Bass()

    hsq_ext = nc.declare_dram_parameter("hsq", [H, SQ], BF16, isOutput=False)
    pa8_ext = nc.declare_dram_parameter("pa8", [S // 2, 2, SQ], F8, isOutput=False)
    wq_ext = nc.declare_dram_parameter("wq", [H, H], BF16, isOutput=False)
    wk_ext = nc.declare_dram_parameter("wk", [H, H], BF16, isOutput=False)
    wv_ext = nc.declare_dram_parameter("wv", [H, H], BF16, isOutput=False)
    wm8_ext = nc.declare_dram_parameter("wm8", [H // 2, 2, H], F8, isOutput=False)
    wg_ext = nc.declare_dram_parameter("wg", [128, NT_H], BF16, isOutput=False)
    out_ext = nc.declare_dram_parameter("out", [SQ, H], BF16, isOutput=True)

    PAIR_GROUPS = [[2 * i, 2 * i + 1] for i in range(N_CORES // 2)]

    MULT = mybir.AluOpType.mult
    ADD = mybir.AluOpType.add
    DR = mybir.MatmulPerfMode.DoubleRow

    with tile.TileContext(nc) as tc:
        with tc.tile_pool(name="persist", bufs=1) as pp:
            # small constants
            identity = pp.tile([128, 128], BF16)
            make_identity(nc, identity)
            ones_row = pp.tile([1, 128], BF16)
            nc.vector.memset(ones_row, G_SCALE)
            wg_sb = pp.tile([128, NT_H], BF16)

            kT_sb = [pp.tile([128, S], BF16, name=f"kT{t}") for t in range(NT_H)]
            qT_sb = [pp.tile([128, SQ], BF16, name=f"qT{t}") for t in range(NT_H)]
            v_sb = [pp.tile([128, H], BF16, name=f"v{t}") for t in range(NT_S)]
            v8_sb = [pp.tile([128, 2, H], F8, name=f"v8_{t}") for t in range(NT_SP)]
            pa8_sb = [
                pp.tile([128, 2, SQ], F8, name=f"pa8_{t}") for t in range(NT_SP)
            ]
            qhatT_sb = [
                pp.tile([128, SQ], BF16, name=f"qhatT{t}") for t in range(NT_H)
            ]
            g_bcast = pp.tile([128, SQ], BF16)
            g_row = pp.tile([1, SQ], BF16)
            rsum_sb = [pp.tile([128, 1], F32, name=f"rsum{t}") for t in range(NT_Q)]

            # ---- stage 1: key-split projections + pair AllGathers ----
            with (
                tc.tile_pool(name="stage1", bufs=1) as s1,
                tc.tile_pool(name="dram_cc", bufs=1, space="DRAM") as dcc,
                tc.tile_pool(name="ps1", bufs=4, space="PSUM") as ps1,
            ):
                hsq_sb = [
                    s1.tile([128, SQ], BF16, name=f"hsq{t}") for t in range(NT_H)
                ]
                wq_sb = [
                    s1.tile([128, H], BF16, name=f"wqs{t}") for t in range(NT_H)
                ]
                wk_sb = [
                    s1.tile([128, H], BF16, name=f"wks{t}") for t in range(NT_H)
                ]
                wv_sb = [
                    s1.tile([128, H], BF16, name=f"wvs{t}") for t in range(NT_H)
                ]
                # DMA priority order: v matmuls consume wv+hsq first
                for t in range(NT_H):
                    nc.sync.dma_start(out=wv_sb[t], in_=wv_ext[t * 128:(t + 1) * 128, :])
                    nc.sync.dma_start(out=hsq_sb[t], in_=hsq_ext[t * 128:(t + 1) * 128, :])
                for t in range(NT_H):
                    nc.sync.dma_start(out=wk_sb[t], in_=wk_ext[t * 128:(t + 1) * 128, :])
                for t in range(NT_H):
                    nc.sync.dma_start(out=wq_sb[t], in_=wq_ext[t * 128:(t + 1) * 128, :])
                nc.sync.dma_start(out=wg_sb, in_=wg_ext[:, :])
                for t in range(NT_SP):
                    nc.sync.dma_start(
                        out=pa8_sb[t], in_=pa8_ext[t * 128:(t + 1) * 128, :, :]
                    )

                vb_in = dcc.tile([SQ, H], BF16)
                vb_out = dcc.tile([S, H], BF16)
                v8b_in = dcc.tile([SQ, H], F8)
                v8b_out = dcc.tile([S, H], F8)
                kb_in = dcc.tile([H, SQ], BF16)
                kb_out = dcc.tile([2 * H, SQ], BF16)
                warm_in = dcc.tile([1, 128], BF16)
                warm_out = dcc.tile([2, 128], BF16)

                # tiny dummy collective at t~0: absorbs the first-collective
                # launch/warmup cost (~20us) so the real gathers run warm
                warm_sb = s1.tile([1, 128], BF16)
                nc.vector.memset(warm_sb, 0.0)
                nc.gpsimd.dma_start(out=warm_in[:, :], in_=warm_sb)
                nc.gpsimd.collective_compute(
                    "AllGather",
                    mybir.AluOpType.bypass,
                    replica_groups=PAIR_GROUPS,
                    ins=[warm_in.opt()],
                    outs=[warm_out.opt()],
                )

                # v own-key half: v[s, d] = sum_hi hsq[hi, s] Wv[hi, d]
                # Each PSUM chunk is drained twice -- bf16 (for ctx/collective)
                # and fp8 (for the DoubleRow mvT) -- so the fp8 AllGather can
                # run first and nothing downstream ever casts behind a
                # collective. Scratch targets (low v_sb / v8_sb slots) are
                # overwritten by the gathered reload later.
                for st in range(NT_SP):
                    for dc in range(NC_H):
                        acc = ps1.tile([128, 512], F32, tag="acc")
                        for hi in range(NT_H):
                            nc.tensor.matmul(
                                acc,
                                hsq_sb[hi][:, st * 128:(st + 1) * 128],
                                wv_sb[hi][:, dc * 512:(dc + 1) * 512],
                                start=(hi == 0),
                                stop=(hi == NT_H - 1),
                            )
                        dstb = v_sb[st][:, dc * 512:(dc + 1) * 512]
                        dst8 = v8_sb[st // 2][:, st % 2, dc * 512:(dc + 1) * 512]
                        if dc == 0:
                            nc.vector.tensor_copy(out=dstb, in_=acc)
                            nc.scalar.copy(out=dst8, in_=acc)
                        else:
                            nc.scalar.copy(out=dstb, in_=acc)
                            nc.vector.tensor_copy(out=dst8, in_=acc)
                        nc.gpsimd.dma_start(
                            out=v8b_in[
                                st * 128:(st + 1) * 128, dc * 512:(dc + 1) * 512
                            ],
                            in_=dst8,
                        )
                        nc.scalar.dma_start(
                            out=vb_in[
                                st * 128:(st + 1) * 128, dc * 512:(dc + 1) * 512
                            ],
                            in_=dstb,
                        )
                nc.gpsimd.collective_compute(
                    "AllGather",
                    mybir.AluOpType.bypass,
                    replica_groups=PAIR_GROUPS,
                    ins=[v8b_in.opt()],
                    outs=[v8b_out.opt()],
                )

                # kT own-key half: kT[do, s] = sum_hi Wk[hi, do] hsq[hi, s]
                # (staged into the left half of kT_sb, overwritten by reload)
                for ho in range(NT_H):
                    for kc in range(NC_Q):
                        acc = ps1.tile([128, 512], F32, tag="acc")
                        for hi in range(NT_H):
                            nc.tensor.matmul(
                                acc,
                                wk_sb[hi][:, ho * 128:(ho + 1) * 128],
                                hsq_sb[hi][:, kc * 512:(kc + 1) * 512],
                                start=(hi == 0),
                                stop=(hi == NT_H - 1),
                            )
                        dst = kT_sb[ho][:, kc * 512:(kc + 1) * 512]
                        if kc == 0:
                            nc.vector.tensor_copy(out=dst, in_=acc)
                        else:
                            nc.scalar.copy(out=dst, in_=acc)
                    nc.scalar.dma_start(
                        out=kb_in[ho * 128:(ho + 1) * 128, :], in_=kT_sb[ho][:, 0:SQ]
                    )
                nc.gpsimd.collective_compute(
                    "AllGather",
                    mybir.AluOpType.bypass,
                    replica_groups=PAIR_GROUPS,
                    ins=[kb_in.opt()],
                    outs=[kb_out.opt()],
                )
                # bf16 v gather last: its result (ctx inputs) is needed last
                nc.gpsimd.collective_compute(
                    "AllGather",
                    mybir.AluOpType.bypass,
                    replica_groups=PAIR_GROUPS,
                    ins=[vb_in.opt()],
                    outs=[vb_out.opt()],
                )

                # reloads in global key order (rank concat), on the sync
                # queue in the same order the collectives complete. The
                # tile_wait_until stamps tell the static scheduler when the
                # collectives realistically finish on hardware, so nothing
                # that depends on these gets ordered ahead of independent
                # work on shared engine queues (stamps are schedule-time
                # only -- execution is still semaphore-driven).
                with tc.tile_wait_until(0.085):
                    for tp in range(NT_SP):
                        for i in range(2):
                            st = 2 * tp + i
                            nc.sync.dma_start(
                                out=v8_sb[tp][:, i, :],
                                in_=v8b_out[st * 128:(st + 1) * 128, :],
                            )
                with tc.tile_wait_until(0.105):
                    for t in range(NT_H):
                        nc.sync.dma_start(
                            out=kT_sb[t][:, 0:SQ],
                            in_=kb_out[t * 128:(t + 1) * 128, :],
                        )
                        nc.sync.dma_start(
                            out=kT_sb[t][:, SQ:S],
                            in_=kb_out[H + t * 128:H + (t + 1) * 128, :],
                        )
                with tc.tile_wait_until(0.125):
                    for st in range(NT_S):
                        nc.sync.dma_start(
                            out=v_sb[st], in_=vb_out[st * 128:(st + 1) * 128, :]
                        )

                # qT over this core's query rows
                for ho in range(NT_H):
                    for qc in range(NC_Q):
                        acc = ps1.tile([128, 512], F32, tag="acc")
                        for hi in range(NT_H):
                            nc.tensor.matmul(
                                acc,
                                wq_sb[hi][:, ho * 128:(ho + 1) * 128],
                                hsq_sb[hi][:, qc * 512:(qc + 1) * 512],
                                start=(hi == 0),
                                stop=(hi == NT_H - 1),
                            )
                        dst = qT_sb[ho][:, qc * 512:(qc + 1) * 512]
                        if qc == 0:
                            nc.vector.tensor_copy(out=dst, in_=acc)
                        else:
                            nc.scalar.copy(out=dst, in_=acc)

                # gates: glog[1, q] = sum_t wg[:, t]^T qT[t]; sigmoid; broadcast
                # (ones_row carries G_SCALE so g_bcast = G_SCALE * sigmoid)
                for qc in range(NC_Q):
                    gacc = ps1.tile([1, 512], F32, tag="gacc", bufs=2)
                    for t in range(NT_H):
                        nc.tensor.matmul(
                            gacc,
                            wg_sb[:, t:t + 1],
                            qT_sb[t][:, qc * 512:(qc + 1) * 512],
                            start=(t == 0),
                            stop=(t == NT_H - 1),
                        )
                    nc.scalar.activation(
                        g_row[:, qc * 512:(qc + 1) * 512],
                        gacc,
                        mybir.ActivationFunctionType.Sigmoid,
                    )
                for qc in range(NC_Q):
                    gb = ps1.tile([128, 512], F32, tag="gb", bufs=2)
                    nc.tensor.matmul(
                        gb,
                        ones_row,
                        g_row[:, qc * 512:(qc + 1) * 512],
                        start=True,
                        stop=True,
                    )
                    nc.vector.tensor_copy(
                        out=g_bcast[:, qc * 512:(qc + 1) * 512], in_=gb
                    )

            # ---- stage 2: mvT, tfT (fp8 DoubleRow), qhatT ----
            with (
                tc.tile_pool(name="stage2", bufs=1) as s2,
                tc.tile_pool(name="ps2", bufs=6, space="PSUM") as ps2,
            ):
                wm8_sb = [
                    s2.tile([128, 2, H], F8, name=f"wm8_{t}") for t in range(NT_DP)
                ]
                mv8_sb = [
                    s2.tile([128, 2, SQ], F8, name=f"mv8_{t}") for t in range(NT_DP)
                ]
                # scalar queue: nothing blocking ahead of it by this point
                # (the sync queue is stuck behind AllGather-gated reloads)
                for t in range(NT_DP):
                    nc.scalar.dma_start(
                        out=wm8_sb[t], in_=wm8_ext[t * 128:(t + 1) * 128, :, :]
                    )

                # mvT[d, q] = sum_k v[k, d] paT[k, q]   (PA_SCALE folded in pa8)
                for qc in range(NC_Q):
                    for d in range(NT_H):
                        acc = ps2.tile([128, 512], F32, tag="acc2")
                        for tp in range(NT_SP):
                            nc.tensor.matmul(
                                acc,
                                v8_sb[tp][:, :, d * 128:(d + 1) * 128],
                                pa8_sb[tp][:, :, qc * 512:(qc + 1) * 512],
                                start=(tp == 0),
                                stop=(tp == NT_SP - 1),
                                perf_mode=DR,
                            )
                        dst = mv8_sb[d // 2][:, d % 2, qc * 512:(qc + 1) * 512]
                        if d % 2 == 0:
                            nc.vector.tensor_copy(out=dst, in_=acc)
                        else:
                            nc.scalar.copy(out=dst, in_=acc)

                # tfT[do, q] = sum_d wm8[d, do] mv8[d, q];
                # qhatT = qT + g_bcast * tfT  (G_SCALE in g_bcast)
                for qc in range(NC_Q):
                    for do in range(NT_H):
                        acc = ps2.tile([128, 512], F32, tag="acc2")
                        for dp in range(NT_DP):
                            nc.tensor.matmul(
                                acc,
                                wm8_sb[dp][:, :, do * 128:(do + 1) * 128],
                                mv8_sb[dp][:, :, qc * 512:(qc + 1) * 512],
                                start=(dp == 0),
                                stop=(dp == NT_DP - 1),
                                perf_mode=DR,
                            )
                        sl = slice(qc * 512, (qc + 1) * 512)
                        tmp = s2.tile([128, 512], BF16, tag="gm_tmp", bufs=3)
                        nc.vector.tensor_tensor(tmp, acc, g_bcast[:, sl], MULT)
                        nc.vector.tensor_tensor(
                            qhatT_sb[do][:, sl], tmp, qT_sb[do][:, sl], ADD
                        )

            # ---- stage 3: per q-tile attention ----
            # logits computed in two [128,1024] halves (2 PSUM banks each,
            # double-buffered) so exp of one half overlaps matmuls of the
            # next; exp is the only ACT-routed op here to keep its queue clear
            with (
                tc.tile_pool(name="stage3", bufs=1) as s3,
                tc.tile_pool(name="ps_logit", bufs=2, space="PSUM") as pslg,
                tc.tile_pool(name="ps_small", bufs=2, space="PSUM") as pssm,
            ):
                for qt in range(NT_Q):
                    qsl = slice(qt * 128, (qt + 1) * 128)
                    probs = s3.tile([128, S], BF16, tag="probs", bufs=2)
                    hsum = [None, None]
                    for half in range(2):
                        lg = pslg.tile([128, 1024], F32, tag="lg")
                        for kk2 in range(2):
                            kk = half * 2 + kk2
                            for d in range(NT_H):
                                nc.tensor.matmul(
                                    lg[:, kk2 * 512:(kk2 + 1) * 512],
                                    qhatT_sb[d][:, qsl],
                                    kT_sb[d][:, kk * 512:(kk + 1) * 512],
                                    start=(d == 0),
                                    stop=(d == NT_H - 1),
                                )
                        hs_t = s3.tile(
                            [128, 1], F32, tag=f"hsum{half}", bufs=2, name=f"hs{half}"
                        )
                        nc.scalar.activation(
                            probs[:, half * 1024:(half + 1) * 1024],
                            lg,
                            mybir.ActivationFunctionType.Exp,
                            accum_out=hs_t,
                        )
                        hsum[half] = hs_t
                    nc.vector.tensor_add(rsum_sb[qt], hsum[0], hsum[1])
                    nc.vector.reciprocal(rsum_sb[qt], rsum_sb[qt])

                    probsT = s3.tile([128, S], BF16, tag="probsT", bufs=2)
                    for g2 in range(2):
                        tp = pssm.tile([128, 1024], BF16, tag="tp")
                        for j in range(8):
                            kt = g2 * 8 + j
                            nc.tensor.transpose(
                                tp[:, j * 128:(j + 1) * 128],
                                probs[:, kt * 128:(kt + 1) * 128],
                                identity,
                            )
                        nc.vector.tensor_copy(
                            out=probsT[:, g2 * 1024:(g2 + 1) * 1024], in_=tp
                        )

                    out_sb = s3.tile([128, H], BF16, tag="out_sb", bufs=2)
                    for dc in range(NC_H):
                        ctx = pssm.tile([128, 512], F32, tag="ctx")
                        for kt in range(NT_S):
                            nc.tensor.matmul(
                                ctx,
                                probsT[:, kt * 128:(kt + 1) * 128],
                                v_sb[kt][:, dc * 512:(dc + 1) * 512],
                                start=(kt == 0),
                                stop=(kt == NT_S - 1),
                            )
                        nc.vector.tensor_scalar_mul(
                            out_sb[:, dc * 512:(dc + 1) * 512], ctx, rsum_sb[qt]
                        )
                        nc.gpsimd.dma_start(
                            out=out_ext[qsl, dc * 512:(dc + 1) * 512],
                            in_=out_sb[:, dc * 512:(dc + 1) * 512],
                        )

    _split_multi_wait_instructions(nc)
    return nc


_cache = {}
last_results = None


def _install_trace_hook_fallback():
    # If BASS_TRACE is set in the environment, run_bass_kernel_spmd imports
    # antenv.axon_hooks, which doesn't exist in bare containers. Provide a
    # stub (no-op hook) so the run degrades to untraced instead of crashing.
    try:
        import antenv.axon_hooks  # noqa: F401
    except ImportError:
        import types

        mod = types.ModuleType("antenv.axon_hooks")
        mod.set_axon_ntff_profile_hook = lambda h: None
        mod.get_axon_ntff_profile_hook = lambda: None
        sys.modules["antenv.axon_hooks"] = mod


def _maybe_reset_device():
    # Recover a wedged axon-tunneled device (NRT_EXEC_UNIT_UNRECOVERABLE
    # persists across processes otherwise). Best effort only.
    try:
        import jax

        try:
            jax.device_put(np.zeros(1, np.float32), jax.devices()[0]).block_until_ready()
            return
        except Exception:
            pass
        import ctypes

        lib = ctypes.CDLL("/opt/axon/libaxon_pjrt.so")
        lib.axon_reset.restype = ctypes.c_int64
        lib.axon_reset()
    except Exception:
        pass


def prepare_in_maps(hidden_states, past_attention, Wq, Wk, Wv, Wm, w_gate):
    hs = np.asarray(hidden_states, dtype=np.float32)
    pa = np.asarray(past_attention, dtype=np.float32)
    Wq = np.asarray(Wq, dtype=np.float32)
    Wk = np.asarray(Wk, dtype=np.float32)
    Wv = np.asarray(Wv, dtype=np.float32)
    Wm = np.asarray(Wm, dtype=np.float32)
    w_gate = np.asarray(w_gate, dtype=np.float32)

    bf = ml_dtypes.bfloat16
    f8 = ml_dtypes.float8_e4m3
    inv_sqrt_h = 1.0 / math.sqrt(H)
    decay = math.exp(-0.5)

    wq_b = np.ascontiguousarray(Wq).astype(bf)
    wk_b = np.ascontiguousarray(Wk * inv_sqrt_h).astype(bf)
    wv_b = np.ascontiguousarray(Wv).astype(bf)
    wm8 = np.ascontiguousarray(
        (Wm * (decay * WM_SCALE))
        .reshape(NT_DP, 2, 128, H)
        .transpose(0, 2, 1, 3)
        .reshape(H // 2, 2, H)
    ).astype(f8)
    wg_b = np.ascontiguousarray(w_gate.reshape(NT_H, 128).T).astype(bf)

    in_maps = []
    hsT_by_batch = [np.ascontiguousarray(hs[b].T).astype(bf) for b in range(B)]
    for c in range(N_CORES):
        b, h = divmod(c, 2)
        hsq = hsT_by_batch[b][:, h * SQ:(h + 1) * SQ]  # [H, SQ] own rows
        paT = pa[b, h * SQ:(h + 1) * SQ, :].T  # [S, SQ] keys x own queries
        pa8 = np.ascontiguousarray(
            (paT * PA_SCALE)
            .reshape(NT_SP, 2, 128, SQ)
            .transpose(0, 2, 1, 3)
            .reshape(S // 2, 2, SQ)
        ).astype(f8)
        in_maps.append(
            {
                "hsq": np.ascontiguousarray(hsq),
                "pa8": pa8,
                "wq": wq_b,
                "wk": wk_b,
                "wv": wv_b,
                "wm8": wm8,
                "wg": wg_b,
            }
        )
    return in_maps


def kernel(hidden_states, past_attention, Wq, Wk, Wv, Wm, w_gate):
    global last_results
    in_maps = prepare_in_maps(
        hidden_states, past_attention, Wq, Wk, Wv, Wm, w_gate
    )

    _install_trace_hook_fallback()
    _maybe_reset_device()
    if "nc" not in _cache:
        _cache["nc"] = build_nc()
    nc = _cache["nc"]

    res = run_bass_kernel_spmd(nc, in_maps, core_ids=list(range(N_CORES)))
    last_results = res

    out = np.empty((B, S, H), dtype=np.float32)
    for c in range(N_CORES):
        b, h = divmod(c, 2)
        out[b, h * SQ:(h + 1) * SQ, :] = res.results[c]["out"].astype(np.float32)
    return out

